# revision 20
# baseline (speedup 1.0000x reference)
"""Trainium2 Bass kernel for nn_LookupTableMy (embedding gathers + LSTM + window dots).

Computation (per sample b):
  e1 = emb[input1[b]]                 # [19, 128]
  h  = LSTM(e1)  (H=384, 19 steps)    # final hidden [384]
  e2 = emb[input2[b]]                 # [20, 128]
  s_j[k] = h[128j:128j+128] . e2[k]   # j=0..2, k=0..19
  rs[n]  = s_0[n] + s_1[n+1] + s_2[n+2]   n=0..17
  ms     = max_n rs[n]
  out    = log_softmax(ms * lin_w[:,0] + lin_b)   # [2]

Sharding: data-parallel over batch: 4096 samples -> 8 cores x 512.
Weights replicated; the embedding table is compacted per core on the host
(each core touches <= 19,968 of the 100k rows; unique rows are packed into
a [20480, 128] f16 table and indices remapped to int16).

v3 design (vs the 426us f32r baseline):
  - dma_gather (gpsimd SWDGE, mlp library) instead of 156 single-row
    indirect DMAs: e1 is 19 transpose-mode gathers (one per timestep,
    512 rows each) that write xT[d, t, b] DIRECTLY -- no PE transposes,
    no DVE copies. e2 is 4 flat gathers (2560 rows each). Transpose-mode
    gathers race across SWDGE queues (shared staging), so e1 stays on
    queue 0; flat e2 gathers spread over queues 1-3.
  - All matmul operands f16 (weights, x, h): rhs streams 2 elem/cycle,
    ~131 ns/MM at N=512 vs ~233 ns for f32r.
  - Gate-major z chunks [128 gate-dims, 512 batch] accumulate in PSUM in
    4 per-gate groups of 3 banks. Bias is pre-seeded into PSUM by a K=1
    matmul (biasT x ones-row), letting one ACTIVATE cover all 3 segments
    of a gate ([128, 1536]): ACT drops from 15x720ns to ~5x1573ns/step.
    o-gate group first so its ACT overlaps the c-path chain.
  - tanh(c) merged over segments; c/h single [128, 3, 512] f16 tiles
    (h slices feed next step's matmuls directly).
"""

import sys
from contextlib import ExitStack

for _p in ("/opt/trn_rl_repo",):
    if _p not in sys.path:
        sys.path.insert(0, _p)

import numpy as np

import concourse.bass as bass
import concourse.tile as tile
import concourse.bacc as bacc
import concourse.mybir as mybir
from concourse import bass_utils

F32 = mybir.dt.float32
F16 = mybir.dt.float16
I16 = mybir.dt.int16
AF = mybir.ActivationFunctionType
ALU = mybir.AluOpType
AX = mybir.AxisListType

V, D, OUT = 100000, 128, 2
H = 3 * D
B, L1, L2 = 4096, 19, 20
NWIN = 18
NCORES = 8
BC = B // NCORES          # 512 samples per core
NB = BC // 128            # 4 batch chunks of 128
NJ = 3                    # hidden segments of 128
VC = 20480                # compact per-core table rows (>= unique count)
N1 = L1 * BC              # e1 gather count
N2 = L2 * BC              # e2 gather count

_cache = {}


def _build():
    """Build (and cache) the Bass program shared by all 8 cores."""
    if "nc" in _cache:
        return _cache["nc"]

    nc = bacc.Bacc(
        "TRN2",
        target_bir_lowering=False,
        debug=False,
        enable_asserts=False,
        num_devices=NCORES,
        num_swdge_queues=4,
    )

    emb_d = nc.dram_tensor("emb", [VC, D], F16, kind="ExternalInput").ap()
    wt_d = nc.dram_tensor("wt", [4, 128, 4 * H], F16, kind="ExternalInput").ap()
    biasc_d = nc.dram_tensor("biasc", [4 * H // 128, 128], F32, kind="ExternalInput").ap()
    lwb_d = nc.dram_tensor("lwb", [1, 4], F32, kind="ExternalInput").ap()
    ix1_d = nc.dram_tensor("ix1", [128, N1 // 16], I16, kind="ExternalInput").ap()
    ix2_d = nc.dram_tensor("ix2", [128, N2 // 16], I16, kind="ExternalInput").ap()
    out_d = nc.dram_tensor("out", [BC, OUT], F32, kind="ExternalOutput").ap()

    with tile.TileContext(nc) as tc, ExitStack() as ctx:
        singles = ctx.enter_context(tc.tile_pool(name="singles", bufs=1))
        psum_rs = ctx.enter_context(tc.tile_pool(name="psum_rs", bufs=4, space="PSUM"))
        psum_z = ctx.enter_context(tc.tile_pool(name="psum_z", bufs=4, space="PSUM"))
        gates = ctx.enter_context(tc.tile_pool(name="gates", bufs=2))
        hcpool = ctx.enter_context(tc.tile_pool(name="hc", bufs=2))
        tmp = ctx.enter_context(tc.tile_pool(name="tmp", bufs=3))
        prodp = ctx.enter_context(tc.tile_pool(name="prodp", bufs=3))
        small = ctx.enter_context(tc.tile_pool(name="small", bufs=1))

        # ---- constants (index tensors first: the gather stream waits on them) ----
        ix1_sb = singles.tile([128, N1 // 16], I16, tag="ix1")
        _c3 = 3 * (BC // 16)
        nc.sync.dma_start(out=ix1_sb[:, :_c3], in_=ix1_d[:, :_c3])
        wt_sb = singles.tile([128, 4, 4 * H], F16, tag="wt")
        nc.sync.dma_start(
            out=wt_sb[:, 0:1, :], in_=wt_d[0:1].rearrange("c p g -> p c g")
        )
        bias_col = singles.tile([128, 4 * H // 128], F32, tag="biascol")
        nc.sync.dma_start(out=bias_col[:], in_=biasc_d.rearrange("g p -> p g"))
        nc.sync.dma_start(out=ix1_sb[:, _c3:], in_=ix1_d[:, _c3:])
        ix2_sb = singles.tile([128, N2 // 16], I16, tag="ix2")
        nc.sync.dma_start(out=ix2_sb[:], in_=ix2_d)
        nc.sync.dma_start(
            out=wt_sb[:, 1:4, :], in_=wt_d[1:4].rearrange("c p g -> p c g")
        )
        lwb_sb = singles.tile([128, 4], F32, tag="lwb")
        nc.sync.dma_start(out=lwb_sb[:], in_=lwb_d.to_broadcast([128, 4]))

        ones128 = singles.tile([128, 128], F16, tag="ones128")
        nc.vector.memset(ones128[:], 1.0)

        xT = singles.tile([128, L1, BC], F16, tag="xT")
        e2T = singles.tile([128, L2, BC], F16, tag="e2T")

        # ---- gather streams ----
        # e1: transpose-mode gathers write xT[d, w, b] directly. Queue 0 only
        # (transpose staging races across queues).
        for w in range(L1):
            out_ap = bass.AP(
                tensor=xT.tensor,
                offset=xT.offset + w * BC,
                ap=[xT.ap[0], [0, 1], [1, BC]],
            )
            nc.gpsimd.dma_gather(
                out_ap=out_ap,
                in_ap=emb_d,
                idxs_ap=ix1_sb[:, w * (BC // 16) : (w + 1) * (BC // 16)],
                num_idxs=BC,
                num_idxs_reg=BC,
                elem_size=D,
                transpose=True,
                single_packet=False,
                queue_num=0,
            )
        # e2: transpose-mode gathers too -- e2T[d, k, b] = emb[idx2[b, k]][d].
        # Same queue-0 staging constraint; 4 calls of 5 k-planes each.
        KB = 5
        for kg in range(L2 // KB):
            nq = KB * BC
            out_ap = bass.AP(
                tensor=e2T.tensor,
                offset=e2T.offset + kg * nq,
                ap=[e2T.ap[0], [0, 1], [1, nq]],
            )
            nc.gpsimd.dma_gather(
                out_ap=out_ap,
                in_ap=emb_d,
                idxs_ap=ix2_sb[:, kg * (nq // 16) : (kg + 1) * (nq // 16)],
                num_idxs=nq,
                num_idxs_reg=nq,
                elem_size=D,
                transpose=True,
                single_packet=False,
                queue_num=0,
            )

        # ---- LSTM steps ----
        # Per-chunk z psum tiles (1 bank each, deep pipelining). Chunk order
        # (i_j, f_j, g_j) per segment then the o chunks: c_j starts as soon as
        # segment j's three gates are done while ACT continues with o; the
        # o ACTs and tanh(c) overlap the next step's x-matmuls.
        CHUNKS = [(q, j) for j in range(NJ) for q in (0, 1, 2)] + [
            (3, j) for j in range(NJ)
        ]

        def emit_step(t, h_prev, c_prev):
            gq = {}
            for q, j in CHUNKS:
                gc = q * NJ + j
                cols = slice(gc * 128, (gc + 1) * 128)
                zq = psum_z.tile([128, BC], F32, tag="z", name=f"z{t}_{gc}")
                nc.tensor.matmul(
                    out=zq[:],
                    lhsT=wt_sb[:, 0, cols],
                    rhs=xT[:, t, :],
                    start=True,
                    stop=(t == 0),
                )
                if t > 0:
                    for kj in range(NJ):
                        nc.tensor.matmul(
                            out=zq[:],
                            lhsT=wt_sb[:, 1 + kj, cols],
                            rhs=h_prev[:, kj, :],
                            start=False,
                            stop=(kj == NJ - 1),
                        )
                g = gates.tile([128, BC], F16, tag=f"g{gc}", name=f"g{t}_{gc}")
                nc.scalar.activation(
                    out=g[:],
                    in_=zq[:],
                    func=AF.Tanh if q == 2 else AF.Sigmoid,
                    bias=bias_col[:, gc : gc + 1],
                )
                gq[(q, j)] = g

            cn = hcpool.tile([128, NJ, BC], F16, tag="c", name=f"c{t}")
            for j in range(NJ):
                gi, gf, gg = gq[(0, j)], gq[(1, j)], gq[(2, j)]
                if t == 0:
                    nc.vector.tensor_tensor(
                        out=cn[:, j, :], in0=gi[:], in1=gg[:], op=ALU.mult
                    )
                else:
                    ig = tmp.tile([128, BC], F16, tag="ig", name=f"ig{t}_{j}")
                    nc.vector.tensor_tensor(
                        out=ig[:], in0=gi[:], in1=gg[:], op=ALU.mult
                    )
                    nc.vector.tensor_tensor(
                        out=cn[:, j, :],
                        in0=gf[:],
                        in1=c_prev[:, j, :],
                        op=ALU.mult,
                    )
                    nc.vector.tensor_tensor(
                        out=cn[:, j, :], in0=cn[:, j, :], in1=ig[:], op=ALU.add
                    )
            tc_t = tmp.tile([128, NJ, BC], F16, tag="tc", name=f"tc{t}")
            nc.scalar.activation(out=tc_t[:], in_=cn[:], func=AF.Tanh)
            hn = hcpool.tile([128, NJ, BC], F16, tag="h", name=f"h{t}")
            for j in range(NJ):
                nc.vector.tensor_tensor(
                    out=hn[:, j, :], in0=gq[(3, j)][:], in1=tc_t[:, j, :],
                    op=ALU.mult
                )
            return hn, cn

        h_t = None
        c_t = None
        for t in range(L1):
            h_t, c_t = emit_step(t, h_t, c_t)

        # ---- window dots in the hT layout (no transposes) ----
        # prod_j[d, k, b] = e2T[d, k, b] * h_j[d, b] (k-split so the PE can
        # start); ones-matrix matmuls sum over d and accumulate the 3 shifted
        # segments of each window into PSUM [128, 512] (b-replicated rows);
        # pairwise-tree max + log-softmax run as full-lane [128, 512] ops.
        prods = []
        KH = L2 // 2
        for j in range(NJ):
            prods.append(prodp.tile([128, L2, BC], F16, tag="prod", name=f"pr{j}"))

        def emit_mul(j, half):
            ks = slice(half * KH, (half + 1) * KH)
            hbc = bass.AP(
                tensor=h_t.tensor,
                offset=h_t.offset + j * BC,
                ap=[h_t.ap[0], [0, KH], [1, BC]],
            )
            nc.vector.tensor_tensor(
                out=prods[j][:, ks, :], in0=e2T[:, ks, :], in1=hbc, op=ALU.mult
            )

        msr = small.tile([128, BC], F32, tag="msr")

        def emit_window(n):
            ps = psum_rs.tile([128, BC], F32, tag="rs", name=f"rs{n}")
            for j in range(NJ):
                nc.tensor.matmul(
                    out=ps[:],
                    lhsT=ones128[:],
                    rhs=prods[j][:, n + j, :],
                    start=(j == 0),
                    stop=(j == NJ - 1),
                )
            if n == 0:
                nc.vector.tensor_copy(out=msr[:], in_=ps[:])
            else:
                nc.vector.tensor_tensor(out=msr[:], in0=msr[:], in1=ps[:],
                                        op=ALU.max)

        # interleave: h0-half muls enable windows 0..7; h1-half muls stream
        # between them so the PE never starves and PSUM banks recycle.
        for j in range(NJ):
            emit_mul(j, 0)
        for n in range(4):
            emit_window(n)
        emit_mul(0, 1)
        for n in range(4, 6):
            emit_window(n)
        emit_mul(1, 1)
        for n in range(6, 8):
            emit_window(n)
        emit_mul(2, 1)
        for n in range(8, NWIN):
            emit_window(n)

        # ---- logits + log-softmax (b on the free axis, lanes replicated) ----
        a0 = small.tile([128, BC], F32, tag="a0")
        a1 = small.tile([128, BC], F32, tag="a1")
        nc.vector.tensor_scalar(out=a0[:], in0=msr[:], scalar1=lwb_sb[:, 0:1],
                                scalar2=lwb_sb[:, 2:3], op0=ALU.mult, op1=ALU.add)
        nc.vector.tensor_scalar(out=a1[:], in0=msr[:], scalar1=lwb_sb[:, 1:2],
                                scalar2=lwb_sb[:, 3:4], op0=ALU.mult, op1=ALU.add)
        mx = small.tile([128, BC], F32, tag="mx")
        nc.vector.tensor_tensor(out=mx[:], in0=a0[:], in1=a1[:], op=ALU.max)
        d0 = small.tile([128, BC], F32, tag="d0")
        d1 = small.tile([128, BC], F32, tag="d1")
        nc.vector.tensor_tensor(out=d0[:], in0=a0[:], in1=mx[:], op=ALU.subtract)
        nc.vector.tensor_tensor(out=d1[:], in0=a1[:], in1=mx[:], op=ALU.subtract)
        e0 = small.tile([128, BC], F32, tag="e0")
        e1 = small.tile([128, BC], F32, tag="e1")
        nc.scalar.activation(out=e0[:], in_=d0[:], func=AF.Exp)
        nc.scalar.activation(out=e1[:], in_=d1[:], func=AF.Exp)
        se = small.tile([128, BC], F32, tag="se")
        nc.vector.tensor_tensor(out=se[:], in0=e0[:], in1=e1[:], op=ALU.add)
        lse = small.tile([128, BC], F32, tag="lse")
        nc.scalar.activation(out=lse[:], in_=se[:], func=AF.Ln)
        outI = small.tile([128, BC, OUT], F32, tag="outI")
        nc.vector.tensor_tensor(out=outI[:, :, 0], in0=d0[:], in1=lse[:],
                                op=ALU.subtract)
        nc.vector.tensor_tensor(out=outI[:, :, 1], in0=d1[:], in1=lse[:],
                                op=ALU.subtract)
        out_flat = bass.AP(
            tensor=out_d.tensor,
            offset=out_d.offset,
            ap=[[BC * OUT, 1], [1, BC * OUT]],
        )
        nc.sync.dma_start(
            out=out_flat,
            in_=outI[0:1, :, :].rearrange("p b c -> p (b c)"),
        )

    nc.compile()
    _cache["nc"] = nc
    return nc


def _wrap16(flat):
    """idx i -> partition i%16, col i//16; replicated to 128 partitions."""
    n = len(flat)
    a = np.zeros((16, n // 16), np.int16)
    a[np.arange(n) % 16, np.arange(n) // 16] = flat.astype(np.int16)
    return np.tile(a, (8, 1))


def kernel(input1, input2, emb, W_ih, W_hh, b_ih, b_hh, lin_w, lin_b, _trace=False):
    input1 = np.ascontiguousarray(np.asarray(input1, dtype=np.int64))
    input2 = np.ascontiguousarray(np.asarray(input2, dtype=np.int64))
    emb = np.asarray(emb, dtype=np.float32)
    W_ih = np.asarray(W_ih, dtype=np.float32)
    W_hh = np.asarray(W_hh, dtype=np.float32)
    b = (np.asarray(b_ih, dtype=np.float32) + np.asarray(b_hh, dtype=np.float32))
    lin_w = np.asarray(lin_w, dtype=np.float32)
    lin_b = np.asarray(lin_b, dtype=np.float32)

    # weight layout: Wfull = [W_ih | W_hh] : [4H, 128+384]; lhsT tiles are
    # Wfull.T reshaped [4 k-chunks, 128, 4H]
    wfull = np.concatenate([W_ih, W_hh], axis=1)          # [1536, 512]
    wt = np.ascontiguousarray(wfull.T.reshape(4, 128, 4 * H).astype(np.float16))
    biasc = np.ascontiguousarray(b.reshape(4 * H // 128, 128))
    lwb = np.ascontiguousarray(
        np.array([[lin_w[0, 0], lin_w[1, 0], lin_b[0], lin_b[1]]], dtype=np.float32)
    )


    nc = _build()

    in_maps = []
    for c in range(NCORES):
        i1 = input1[c * BC : (c + 1) * BC]                # [512, 19]
        i2 = input2[c * BC : (c + 1) * BC]                # [512, 20]
        ids = np.unique(np.concatenate([i1.ravel(), i2.ravel()]))
        assert len(ids) <= VC, len(ids)
        embc = np.zeros((VC, D), np.float16)
        embc[: len(ids)] = emb[ids].astype(np.float16)
        c1 = np.searchsorted(ids, i1)                     # [512, 19]
        c2 = np.searchsorted(ids, i2)                     # [512, 20]
        # e1 flat order per wave: i = b (= cb*128+p)
        ix1 = np.concatenate(
            [_wrap16(c1[:, w]) for w in range(L1)], axis=1
        )
        # e2 transposed-gather flat order: i = k*BC + b
        ix2 = _wrap16(c2.T.ravel())
        in_maps.append(
            {
                "emb": embc,
                "wt": wt,
                "biasc": biasc,
                "lwb": lwb,
                "ix1": np.ascontiguousarray(ix1),
                "ix2": np.ascontiguousarray(ix2),
            }
        )

    res = bass_utils.run_bass_kernel_spmd(
        nc, in_maps, core_ids=list(range(NCORES)), trace=_trace
    )
    if _trace:
        kernel.last_results = res
    out = np.concatenate([res.results[c]["out"] for c in range(NCORES)], axis=0)
    return out


if __name__ == "__main__":
    rng = np.random.default_rng(0)
    inputs = {
        "input1": rng.integers(0, V, (B, L1), dtype=np.int32),
        "input2": rng.integers(0, V, (B, L2), dtype=np.int32),
        "emb": rng.standard_normal((V, D), dtype=np.float32),
        "W_ih": (rng.standard_normal((4 * H, D), dtype=np.float32) * 0.05),
        "W_hh": (rng.standard_normal((4 * H, H), dtype=np.float32) * 0.05),
        "b_ih": (rng.standard_normal(4 * H).astype(np.float32) * 0.05),
        "b_hh": (rng.standard_normal(4 * H).astype(np.float32) * 0.05),
        "lin_w": rng.standard_normal((OUT, 1), dtype=np.float32),
        "lin_b": rng.standard_normal(OUT).astype(np.float32),
    }
    out = kernel(**inputs)
    print(out.shape, out[:2])
